# revision 45
# baseline (speedup 1.0000x reference)
"""Trainium2 Bass kernel for a causal attention block (QKV + RoPE + attention + out-proj).

Structure: TP=2 over heads x DP=4 over batch; per core: [T=2048] tokens,
8 heads, 512 features. f32r matmuls everywhere (measured 1 cyc/row on HW for
moving>=256; bf16 measured SLOWER).

HW-measurement-driven deltas vs the phase-separated baseline:
  - AV matmuls lag 3 steps behind their exp (deque pipeline) so the PE never
    stalls on ACT latency (measured cross-engine round trip ~570ns vs the
    100ns the cost model assumes)
  - rope entirely on DVE: the Pool/GPSIMD engine measured ~2.4-3x slower
    than nominal for tensor ops (software impl), and any Pool op on the
    phase-1 path cost ~100us
  - softmax normalize restructured (fast_norm): ctx evicted to SBUF (on ACT,
    which idles at pair boundaries) immediately after the last AV so the
    pctx PSUM bank frees without waiting for the 4-hop normalize chain;
    the normalize multiply reads the row-broadcast straight from PSUM
  - out-proj eviction on ACT (obuf_act), mask multiply on DVE after exp
  - big weight/activation DMAs split across HW queues (single queue is only
    ~120 GB/s; ~220+ GB/s aggregate)
  - SBUF/PSUM pools merged per phase (each tile_pool scope costs ~1.1us of
    all-engine barrier per repetition)
"""
import numpy as np
from collections import deque

B, T, D = 4, 2048, 1024
H_TOTAL, HD = 16, 64
N_CORES = 8
H = H_TOTAL // 2        # heads per core (TP rank)
FS = H * HD             # 512 sharded q/k/v features per core
HD1 = HD + 1            # head dim + ones column
KT = T // 128           # 16 key tiles
QB = T // 512           # 4 query blocks
FC = FS // 128          # 4 feature chunks
DC = D // 128           # 8 d_model chunks
TC = T // 128           # 16 token chunks
SCALE = 1.0 / np.sqrt(HD)

_CACHE = {}


def _split_waits(nc, mybir, maxw=1):
    """This env's walrus encodes at most one sem wait per instruction; move
    extra waits onto same-engine NoOp carriers inserted just before."""
    import copy
    eng_map = {
        mybir.EngineType.PE: nc.tensor,
        mybir.EngineType.DVE: nc.vector,
        mybir.EngineType.Activation: nc.scalar,
        mybir.EngineType.Pool: nc.gpsimd,
        mybir.EngineType.SP: nc.sync,
    }
    protos = {}

    def proto(engine):
        if engine not in protos:
            mi = eng_map[engine].nop(nofuse=True).ins
            for blk in nc.m.functions[0].blocks:
                insts = list(blk.instructions)
                if insts and insts[-1].name == mi.name:
                    blk.instructions = insts[:-1]
                    break
            protos[engine] = mi
        return protos[engine]

    ctr = 0
    for blk in nc.m.functions[0].blocks:
        out = []
        changed = False
        for inst in blk.instructions:
            si = inst.sync_info
            waits = list(si.on_wait) if si and si.on_wait else []
            if len(waits) > maxw and getattr(inst, "engine", None) is not None:
                head, keep = waits[:-maxw], waits[-maxw:]
                for i in range(0, len(head), maxw):
                    nop = copy.deepcopy(proto(inst.engine))
                    ctr += 1
                    nop.name = f"I-wsplit-{ctr}"
                    nop.sync_info = mybir.SyncInfo(on_wait=head[i:i + maxw], on_update=[])
                    out.append(nop)
                si.on_wait = keep
                changed = True
            out.append(inst)
        if changed:
            blk.instructions = out
    return


DEFAULT_FLAGS = frozenset({"lag3", "dma_split", "rope_all_dve", "fast_norm",
                           "obuf_act", "evict_act"})


def _build_nc(R=1, phases=(1, 2, 3), flags=DEFAULT_FLAGS):
    import concourse.bass as bass
    import concourse.mybir as mybir
    import concourse.tile as tile

    lag = (4 if "lag4" in flags else 3 if "lag3" in flags
           else 2 if "lag2" in flags else 1)

    f32 = mybir.dt.float32
    f32r = mybir.dt.float32r
    bf16 = mybir.dt.bfloat16
    Exp = mybir.ActivationFunctionType.Exp

    nc = bass.Bass("TRN2", target_bir_lowering=False, debug=False)
    xT = nc.dram_tensor("xT", [D, T], f32r, kind="ExternalInput").ap()
    wqT = nc.dram_tensor("wqT", [D, FS], f32r, kind="ExternalInput").ap()
    wkT = nc.dram_tensor("wkT", [D, FS], f32r, kind="ExternalInput").ap()
    wvT = nc.dram_tensor("wvT", [D, FS], f32r, kind="ExternalInput").ap()
    woT = nc.dram_tensor("woT", [FS, D], f32r, kind="ExternalInput").ap()
    ropeC = nc.dram_tensor("ropeC", [128, T], f32, kind="ExternalInput").ap()
    ropeS = nc.dram_tensor("ropeS", [128, T], f32, kind="ExternalInput").ap()
    triM = nc.dram_tensor("triM", [128, 128], bf16, kind="ExternalInput").ap()
    ident = nc.dram_tensor("ident", [128, 128], bf16, kind="ExternalInput").ap()
    tri = nc.dram_tensor("tri", [128, 128], f32r, kind="ExternalInput").ap()
    triMr = nc.dram_tensor("triMr", [128, 128], f32r, kind="ExternalInput").ap()
    identr = nc.dram_tensor("identr", [128, 128], f32r, kind="ExternalInput").ap()
    out_dt = bf16 if "out_bf16" in flags else f32
    out = nc.dram_tensor("out", [T, D], out_dt, kind="ExternalOutput").ap()

    with tile.TileContext(nc) as tc:
      for _rep in range(R):
        with tc.tile_pool(name="persist", bufs=1) as persist:
            qT = persist.tile([128, FC, T], f32r)      # [feat, tok], 2 heads/chunk
            kT = persist.tile([128, FC, T], f32r)
            v_ext = persist.tile([128, TC, H * HD1], f32r)  # [tok, 8*(64+1)]

            # ---------------- phase 1: projections ----------------
            if 1 in phases:
              with tc.tile_pool(name="xw", bufs=1) as xw:
                xT_s = xw.tile([128, DC, T], f32r)
                xr = xT.rearrange("(c p) t -> p c t", p=128)
                # startup order: first token chunk + first weight stripe reach
                # SBUF before the bulk of xT so the PE starts at ~10us.
                # split across 2 queues.
                if "dma_split" in flags:
                    nc.sync.dma_start(xT_s[:, 0:4, 0:512], xr[:, 0:4, 0:512])
                    nc.sync.dma_start(xT_s[:, 4:8, 0:512], xr[:, 4:8, 0:512])
                else:
                    nc.sync.dma_start(xT_s[:, :, 0:512], xr[:, :, 0:512])

                # q, k first (feeds the rope DVE stream early), then v.
                # single SBUF + single PSUM pool: fewer scope barriers
                with tc.tile_pool(name="p1b", bufs=1) as p1b, \
                     tc.tile_pool(name="pp1", bufs=4, space="PSUM") as pp1:
                    wvp = wstripe = ropep = ptmp = p1b
                    pps_v = pps = pp1
                    stripes = {}
                    wst0 = wstripe.tile([128, DC, 128], f32r, tag="wst", bufs=2)
                    nc.sync.dma_start(
                        wst0, wqT[:, 0:128].rearrange("(c p) m -> p c m", p=128)
                    )
                    stripes[(0, 0)] = wst0
                    rc = ropep.tile([128, T], f32)
                    rs = ropep.tile([128, T], f32)
                    nc.sync.dma_start(rc, ropeC)
                    nc.sync.dma_start(rs, ropeS)
                    for tb in range(1, QB):
                        tsl = slice(tb * 512, (tb + 1) * 512)
                        if "dma_split" in flags:
                            nc.sync.dma_start(xT_s[:, 0:4, tsl], xr[:, 0:4, tsl])
                            nc.sync.dma_start(xT_s[:, 4:8, tsl], xr[:, 4:8, tsl])
                        else:
                            nc.sync.dma_start(xT_s[:, :, tsl], xr[:, :, tsl])
                    wvT_s = wvp.tile([128, DC, FS], f32r)
                    # ones columns of v_ext (head h col 64 at 65h+64)
                    nc.vector.memset(
                        v_ext.rearrange("p c (h e) -> p c h e", e=HD1)[:, :, :, HD:HD1].bitcast(f32),
                        1.0,
                    )

                    for dst_i, (dst, w_dram) in enumerate(((qT, wqT), (kT, wkT))):
                        for fc in range(FC):
                            if (dst_i, fc) in stripes:
                                wst = stripes[(dst_i, fc)]
                            else:
                                wst = wstripe.tile([128, DC, 128], f32r, tag="wst", bufs=2)
                                nc.sync.dma_start(
                                    wst,
                                    w_dram[:, fc * 128:(fc + 1) * 128].rearrange("(c p) m -> p c m", p=128),
                                )
                            for tb in range(QB):
                                tsl = slice(tb * 512, (tb + 1) * 512)
                                psum = pps.tile([128, 512], f32, tag="pqk")
                                for kc in range(DC):
                                    nc.tensor.matmul(
                                        psum,
                                        wst[:, kc, :],
                                        xT_s[:, kc, tsl],
                                        start=(kc == 0), stop=(kc == DC - 1),
                                    )
                                # rope: evict psum into the destination on
                                # the (idle) ACT, then the rotate-half muls
                                # split across DVE and Pool; final add on Pool.
                                tmp = ptmp.tile([128, 512], f32r, tag="ropetmp", bufs=2)
                                dsl = dst[:, fc, tsl]
                                nc.scalar.copy(dsl, psum)
                                if "no_rope" in flags:
                                    pass  # timing probe only (wrong numerics)
                                else:
                                    if "rope_all_dve" in flags:
                                        small_engs = (nc.vector, nc.vector)
                                    else:
                                        small_engs = (nc.vector, nc.gpsimd)
                                    for eng, hb in zip(small_engs, (0, 64)):
                                        eng.tensor_mul(
                                            out=tmp[hb:hb + 32, :],
                                            in0=dsl[hb + 32:hb + 64],
                                            in1=rs[hb + 32:hb + 64, tsl],
                                        )
                                        eng.tensor_mul(
                                            out=tmp[hb + 32:hb + 64, :],
                                            in0=dsl[hb:hb + 32],
                                            in1=rs[hb:hb + 32, tsl],
                                        )
                                    nc.vector.tensor_mul(out=dsl, in0=dsl, in1=rc[:, tsl])
                                    add_eng = (nc.gpsimd if "rope_pool" in flags
                                               and "rope_all_dve" not in flags else nc.vector)
                                    add_eng.tensor_add(out=dsl, in0=dsl, in1=tmp)

                    # v: out tile [tok 128, feat 512] = x @ Wv^T
                    # (wv DMA deferred here so startup bandwidth goes to q/k;
                    # split per-d_model-chunk so the 8 queues all pull and the
                    # kc-th matmul only waits for its own chunk)
                    wvr = wvT.rearrange("(c p) m -> p c m", p=128)
                    if "dma_split" in flags:
                        for c in range(DC):
                            nc.sync.dma_start(wvT_s[:, c:c + 1, :], wvr[:, c:c + 1, :])
                    else:
                        nc.sync.dma_start(wvT_s, wvr)
                    for tcv in range(TC):
                        psum = pps_v.tile([128, FS], f32, tag="pv")
                        for kc in range(DC):
                            nc.tensor.matmul(
                                psum,
                                xT_s[:, kc, tcv * 128:(tcv + 1) * 128],
                                wvT_s[:, kc, :],
                                start=(kc == 0), stop=(kc == DC - 1),
                            )
                        nc.scalar.copy(
                            v_ext.rearrange("p c (h e) -> p c h e", e=HD1)[:, tcv, :, 0:HD],
                            psum.rearrange("p (h e) -> p h e", e=HD),
                        )

            # ---------------- phase 2: attention ----------------
            _ctx_cm = tc.tile_pool(name="ctx", bufs=1)
            ctxp = _ctx_cm.__enter__()
            ctxT = ctxp.tile([128, FC, T], f32r)       # normalized ctx.T
            if 2 in phases:
              with tc.tile_pool(name="p2b", bufs=1) as p2b, \
                 tc.tile_pool(name="pp2", bufs=1, space="PSUM") as pp2:
                maskp = nrm = wop = obuf = p2b
                attnp = p2b
                ps_s = ps_x = ps_c = pp2
                nbuf_at = max(6, lag + 3)
                nbuf_px = 1 if "ps_x1" in flags else 2
                if "ident_mask" in flags:
                    dt_m = f32r if "imask_f32r" in flags else bf16
                    trim = maskp.tile([128, 128], dt_m)
                    nc.sync.dma_start(trim, triMr if dt_m is f32r else triM)
                    idt = maskp.tile([128, 128], dt_m)
                    nc.sync.dma_start(idt, identr if dt_m is f32r else ident)
                else:
                    trim = maskp.tile([128, 128], f32r)
                    nc.sync.dma_start(trim, tri)
                ones64 = maskp.tile([1, 64], f32r)
                nc.vector.memset(ones64.bitcast(f32), 1.0)
                woT_s = wop.tile([128, FC, D], f32r)
                wor = woT.rearrange("(c p) o -> p c o", p=128)
                if "dma_split" in flags:
                    for c in range(FC):
                        nc.sync.dma_start(woT_s[:, c:c + 1, :], wor[:, c:c + 1, :])
                else:
                    nc.sync.dma_start(woT_s, wor)

                # qb outer; two heads' chains interleaved to fill pipeline
                # bubbles; out-projection for this qb's tokens follows so it
                # overlaps the next qb's attention.
                wide = {}  # tco -> accumulating [128, 1024] out tile

                def emit_outproj_chunk(pqb, i):
                    tco = 4 * pqb + i // 2
                    ob = i % 2
                    osl = slice(ob * 512, (ob + 1) * 512)
                    psum = ps_x.tile([128, 512], f32, tag="px", bufs=nbuf_px)
                    for fc in range(FC):
                        nc.tensor.matmul(
                            psum,
                            ctxT[:, fc, tco * 128:(tco + 1) * 128],
                            woT_s[:, fc, osl],
                            start=(fc == 0), stop=(fc == FC - 1),
                        )
                    if "wide_out" in flags:
                        if tco not in wide:
                            ow_t = obuf.tile([128, D], f32, tag="ow", bufs=4)
                            wide[tco] = ow_t
                        ow = wide[tco]
                        cp = nc.scalar.copy if "obuf_act" in flags else (
                            lambda o, p: nc.vector.tensor_copy(out=o, in_=p))
                        cp(ow[:, osl], psum)
                        if ob == 1 and "no_out_dma" not in phases:
                            nc.sync.dma_start(out[tco * 128:(tco + 1) * 128, :], ow)
                            del wide[tco]
                        return
                    if "dma_psum" in flags:
                        if "no_out_dma" not in phases:
                            nc.sync.dma_start(out[tco * 128:(tco + 1) * 128, osl], psum)
                        else:
                            ot = obuf.tile([128, 512], f32, tag="ot", bufs=4)
                            nc.vector.tensor_copy(out=ot, in_=psum)
                    else:
                        ot = obuf.tile([128, 512], out_dt, tag="ot", bufs=4)
                        if "obuf_act" in flags:
                            nc.scalar.copy(ot, psum)
                        else:
                            nc.vector.tensor_copy(out=ot, in_=psum)
                        if "no_out_dma" not in phases:
                            nc.sync.dma_start(out[tco * 128:(tco + 1) * 128, osl], ot)

                prev_qb = None
                for qb in range(QB):
                    qsl = slice(qb * 512, (qb + 1) * 512)
                    nkt = 4 * qb + 4
                    for hp in range(H // 2):
                        pair = (2 * hp, 2 * hp + 1)
                        pctx = {}
                        for h in pair:
                            pctx_h = ps_c.tile([HD1, 512], f32, tag=f"pctx{h % 2}")
                            pctx[h] = pctx_h

                        def emit_av(h, kt2, scs, at):
                            for half in range(2):
                                kt = 2 * kt2 + half
                                sc = scs[half]
                                hsl = slice(half * 512 + sc, (half + 1) * 512)
                                nc.tensor.matmul(
                                    pctx[h][:, sc:],
                                    v_ext[:, kt, h * HD1:(h + 1) * HD1],
                                    at[:, hsl],
                                    start=(kt == 0), stop=(kt == nkt - 1),
                                    skip_group_check=True,
                                )

                        pend = deque()  # (h, kt2, scs, at) awaiting AV
                        for kt2 in range(nkt // 2):
                            for h in pair:
                                chunk, po = h // 2, 64 * (h % 2)
                                qh = qT[po:po + 64, chunk, :]
                                kh = kT[po:po + 64, chunk, :]
                                # two kt tiles share a 2-bank psum: ONE exp
                                # covers both halves (valid column ranges)
                                ps = ps_s.tile([128, 1024], f32, tag="ps", bufs=2)
                                at = attnp.tile([128, 1024], f32r, tag="at", bufs=nbuf_at)
                                scs = []
                                for half in range(2):
                                    kt = 2 * kt2 + half
                                    sc = max(0, 128 * (kt - 4 * qb))
                                    scs.append(sc)
                                    hsl = slice(half * 512 + sc, (half + 1) * 512)
                                    diag = kt >= 4 * qb
                                    use_imask = "ident_mask" in flags and diag
                                    nc.tensor.matmul(
                                        ps[:, hsl],
                                        kh[:, kt * 128:(kt + 1) * 128],
                                        qh[:, qb * 512 + sc:(qb + 1) * 512],
                                        start=True, stop=not use_imask,
                                        skip_group_check=True,
                                    )
                                    if use_imask:
                                        # accumulate the -1e9 causal triangle
                                        # into the diagonal 128-col segment:
                                        # I^T @ triM == triM
                                        msl = slice(half * 512 + sc, half * 512 + sc + 128)
                                        nc.tensor.matmul(
                                            ps[:, msl], idt, trim,
                                            start=False, stop=True,
                                            skip_group_check=True,
                                        )
                                while len(pend) > lag:
                                    emit_av(*pend.popleft())
                                # exp over the union width (equal 2-seg AP);
                                # half B's extra low columns are never read.
                                sc0 = scs[0]
                                if sc0 == 0:
                                    nc.scalar.activation(at, ps, Exp, scale=float(SCALE))
                                else:
                                    seg_o = at.rearrange("p (s c) -> p s c", s=2)[:, :, sc0:]
                                    seg_i = ps.rearrange("p (s c) -> p s c", s=2)[:, :, sc0:]
                                    nc.scalar.activation(seg_o, seg_i, Exp, scale=float(SCALE))
                                if "ident_mask" not in flags:
                                    mask_eng = nc.gpsimd if "pool_mask" in flags else nc.vector
                                    for half in range(2):
                                        kt = 2 * kt2 + half
                                        if kt >= 4 * qb:
                                            sc = scs[half]
                                            msl = slice(half * 512 + sc, half * 512 + sc + 128)
                                            mask_eng.tensor_mul(
                                                out=at[:, msl], in0=at[:, msl], in1=trim)
                                pend.append((h, kt2, scs, at))
                        while pend:
                            emit_av(*pend.popleft())

                        # reciprocals right after the last AV; the previous
                        # qb's out-projection matmuls keep the PE busy while
                        # they complete.
                        rrows = {}
                        ctxus = {}
                        for h in pair:
                            rrow = nrm.tile([1, 512], f32r, tag="rrow", bufs=3)
                            with nc.allow_low_precision(reason="recip feeds PE bcast"):
                                nc.vector.reciprocal(rrow, pctx[h][HD:HD1, :])
                            rrows[h] = rrow
                            if "fast_norm" in flags:
                                # evict unnormalized ctx now so the pctx bank
                                # frees for the next pair's first AV without
                                # waiting on the whole normalize chain
                                ctxu = nrm.tile([HD, 512], f32, tag="ctxu", bufs=3)
                                if "evict_act" in flags:
                                    nc.scalar.copy(ctxu, pctx[h][0:HD, :])
                                else:
                                    nc.vector.tensor_copy(out=ctxu, in_=pctx[h][0:HD, :])
                                ctxus[h] = ctxu
                        if 3 in phases and prev_qb is not None:
                            emit_outproj_chunk(prev_qb, 2 * hp)
                            emit_outproj_chunk(prev_qb, 2 * hp + 1)
                        for h in pair:
                            chunk, po = h // 2, 64 * (h % 2)
                            rbp = ps_x.tile([64, 512], f32, tag="px", bufs=nbuf_px)
                            nc.tensor.matmul(rbp, ones64, rrows[h], start=True, stop=True)
                            if "fast_norm" in flags:
                                # SBUF ctx * PSUM broadcast: one psum operand
                                nc.vector.tensor_mul(
                                    out=ctxT[po:po + 64, chunk, qsl],
                                    in0=ctxus[h],
                                    in1=rbp,
                                )
                            else:
                                rb = nrm.tile([64, 512], f32, tag="rb", bufs=3)
                                nc.vector.tensor_copy(out=rb, in_=rbp)
                                nc.vector.tensor_mul(
                                    out=ctxT[po:po + 64, chunk, qsl],
                                    in0=pctx[h][0:HD, :],
                                    in1=rb,
                                )
                    prev_qb = qb
                if 3 in phases:
                    for i in range(8):
                        emit_outproj_chunk(QB - 1, i)
            _ctx_cm.__exit__(None, None, None)

    _split_waits(nc, mybir)
    return nc


def _make_runner(nc, n_cores):
    """Build the shard_map-jitted PJRT executable once (reusable across calls)."""
    import jax
    import concourse.mybir as mybir
    from jax.sharding import Mesh, PartitionSpec
    from jax.experimental.shard_map import shard_map
    from concourse import bass2jax as b2j

    b2j.install_neuronx_cc_hook()
    partition_name = nc.partition_id_tensor.name if nc.partition_id_tensor else None
    in_names, out_names, out_avals = [], [], []
    for alloc in nc.m.functions[0].allocations:
        if not isinstance(alloc, mybir.MemoryLocationSet):
            continue
        name = alloc.memorylocations[0].name
        if alloc.kind == "ExternalInput":
            if name != partition_name:
                in_names.append(name)
        elif alloc.kind == "ExternalOutput":
            out_names.append(name)
            out_avals.append(
                jax.core.ShapedArray(tuple(alloc.tensor_shape), mybir.dt.np(alloc.dtype))
            )
    all_in_names = list(in_names) + list(out_names)
    if partition_name is not None:
        all_in_names.append(partition_name)

    def _body(*args):
        operands = list(args)
        if partition_name is not None:
            operands.append(b2j.partition_id_tensor())
        return tuple(b2j._bass_exec_p.bind(
            *operands,
            out_avals=tuple(out_avals),
            in_names=tuple(all_in_names),
            out_names=tuple(out_names),
            lowering_input_output_aliases=(),
            sim_require_finite=True,
            sim_require_nnan=True,
            nc=nc,
        ))

    devices = jax.devices()[:n_cores]
    mesh = Mesh(np.asarray(devices), ("core",))
    n_in = len(in_names) + len(out_names)
    fn = jax.jit(
        shard_map(
            _body, mesh=mesh,
            in_specs=(PartitionSpec("core"),) * n_in,
            out_specs=(PartitionSpec("core"),) * len(out_names),
            check_rep=False,
        ),
        keep_unused=True,
    )

    def stage(in_maps):
        import jax as _jax
        per_core = [[np.asarray(m[name]) for name in in_names] for m in in_maps]
        concat_in = [
            np.concatenate([per_core[c][i] for c in range(n_cores)], axis=0)
            for i in range(len(in_names))
        ]
        concat_zeros = [
            np.zeros((n_cores * a.shape[0], *a.shape[1:]), a.dtype) for a in out_avals
        ]
        return [_jax.device_put(a) for a in concat_in + concat_zeros]

    def call_staged(staged):
        import jax as _jax
        out_arrs = fn(*staged)
        _jax.block_until_ready(out_arrs)
        return out_arrs

    def call(in_maps):
        out_arrs = call_staged(stage(in_maps))
        return [
            {name: np.asarray(out_arrs[i]).reshape(n_cores, *out_avals[i].shape)[c]
             for i, name in enumerate(out_names)}
            for c in range(n_cores)
        ]

    call.stage = stage
    call.call_staged = call_staged
    return call


def _host_tables():
    # rope tables in the permuted ([even dims | odd dims] per head) layout:
    # rows 0:32 -> freq j (x1 of head A), 32:64 -> freq j (x2 of head A), repeat.
    j = np.arange(32, dtype=np.float64)
    inv = 1.0 / (10000.0 ** (2.0 * j / HD))
    t = np.arange(T, dtype=np.float64)
    ang = np.outer(inv, t)                      # [32, T]
    c32 = np.cos(ang).astype(np.float32)
    s32 = np.sin(ang).astype(np.float32)
    ropeC = np.concatenate([c32, c32, c32, c32], axis=0)          # [128, T]
    ropeS = np.concatenate([s32, -s32, s32, -s32], axis=0)        # [128, T]
    # additive causal mask for the [128,128] diagonal tile:
    # 0 where key p <= query f (valid), -1e9 otherwise
    import ml_dtypes
    p = np.arange(128)[:, None]
    f = np.arange(128)[None, :]
    triM = np.where(p <= f, 0.0, -1e9).astype(ml_dtypes.bfloat16)
    ident = np.eye(128).astype(ml_dtypes.bfloat16)
    tri = (p <= f).astype(np.float32)
    triMr = np.where(p <= f, 0.0, -1e9).astype(np.float32)
    identr = np.eye(128, dtype=np.float32)
    return ropeC, ropeS, triM, ident, tri, triMr, identr


def _perm_rows():
    # per head: [even dims, odd dims]
    perm = []
    for h in range(H):
        base = h * HD
        perm.extend(base + np.arange(0, HD, 2))
        perm.extend(base + np.arange(1, HD, 2))
    return np.asarray(perm)


def _in_maps(x, Wq, Wk, Wv, Wo):
    ropeC, ropeS, triM, ident, tri, triMr, identr = _host_tables()
    perm = _perm_rows()
    in_maps = []
    for c in range(N_CORES):
        b, r = c // 2, c % 2
        rows = slice(r * FS, (r + 1) * FS)
        in_maps.append({
            "xT": np.ascontiguousarray(x[b].T),
            "wqT": np.ascontiguousarray(Wq[rows][perm].T),
            "wkT": np.ascontiguousarray(Wk[rows][perm].T),
            "wvT": np.ascontiguousarray(Wv[rows].T),
            "woT": np.ascontiguousarray(Wo[:, rows].T),
            "ropeC": ropeC,
            "ropeS": ropeS,
            "triM": triM,
            "ident": ident,
            "tri": tri,
            "triMr": triMr,
            "identr": identr,
        })
    return in_maps


def kernel(x, Wq, Wk, Wv, Wo):
    x = np.asarray(x, dtype=np.float32)
    Wq = np.asarray(Wq, dtype=np.float32)
    Wk = np.asarray(Wk, dtype=np.float32)
    Wv = np.asarray(Wv, dtype=np.float32)
    Wo = np.asarray(Wo, dtype=np.float32)

    if "runner" not in _CACHE:
        nc = _build_nc()
        _CACHE["runner"] = _make_runner(nc, N_CORES)
    call = _CACHE["runner"]

    results = call(_in_maps(x, Wq, Wk, Wv, Wo))
    out = np.empty((B, T, D), dtype=np.float32)
    for b in range(B):
        out[b] = (np.asarray(results[2 * b]["out"], dtype=np.float32)
                  + np.asarray(results[2 * b + 1]["out"], dtype=np.float32))
    return out


# revision 51
# speedup vs baseline: 1.0528x; 1.0528x over previous
"""Trainium2 Bass kernel for a causal attention block (QKV + RoPE + attention + out-proj).

Structure: TP=2 over heads x DP=4 over batch; per core: [T=2048] tokens,
8 heads, 512 features. f32r matmuls everywhere (measured 1 cyc/row on HW for
moving>=256; bf16 measured SLOWER).

HW-measurement-driven deltas vs the phase-separated baseline:
  - AV matmuls lag 3 steps behind their exp (deque pipeline) so the PE never
    stalls on ACT latency (measured cross-engine round trip ~570ns vs the
    100ns the cost model assumes)
  - rope entirely on DVE: the Pool/GPSIMD engine measured ~2.4-3x slower
    than nominal for tensor ops (software impl), and any Pool op on the
    phase-1 path cost ~100us
  - softmax normalize restructured (fast_norm): ctx evicted to SBUF (on ACT,
    which idles at pair boundaries) immediately after the last AV so the
    pctx PSUM bank frees without waiting for the 4-hop normalize chain;
    the normalize multiply reads the row-broadcast straight from PSUM
  - out-proj eviction on ACT (obuf_act), mask multiply on DVE after exp
  - big weight/activation DMAs split across HW queues (single queue is only
    ~120 GB/s; ~220+ GB/s aggregate)
  - SBUF/PSUM pools merged per phase (each tile_pool scope costs ~1.1us of
    all-engine barrier per repetition)
"""
import numpy as np
from collections import deque

B, T, D = 4, 2048, 1024
H_TOTAL, HD = 16, 64
N_CORES = 8
H = H_TOTAL // 2        # heads per core (TP rank)
FS = H * HD             # 512 sharded q/k/v features per core
HD1 = HD + 1            # head dim + ones column
KT = T // 128           # 16 key tiles
QB = T // 512           # 4 query blocks
FC = FS // 128          # 4 feature chunks
DC = D // 128           # 8 d_model chunks
TC = T // 128           # 16 token chunks
SCALE = 1.0 / np.sqrt(HD)

_CACHE = {}


def _split_waits(nc, mybir, maxw=1):
    """This env's walrus encodes at most one sem wait per instruction; move
    extra waits onto same-engine NoOp carriers inserted just before."""
    import copy
    eng_map = {
        mybir.EngineType.PE: nc.tensor,
        mybir.EngineType.DVE: nc.vector,
        mybir.EngineType.Activation: nc.scalar,
        mybir.EngineType.Pool: nc.gpsimd,
        mybir.EngineType.SP: nc.sync,
    }
    protos = {}

    def proto(engine):
        if engine not in protos:
            mi = eng_map[engine].nop(nofuse=True).ins
            for blk in nc.m.functions[0].blocks:
                insts = list(blk.instructions)
                if insts and insts[-1].name == mi.name:
                    blk.instructions = insts[:-1]
                    break
            protos[engine] = mi
        return protos[engine]

    ctr = 0
    for blk in nc.m.functions[0].blocks:
        out = []
        changed = False
        for inst in blk.instructions:
            si = inst.sync_info
            waits = list(si.on_wait) if si and si.on_wait else []
            if len(waits) > maxw and getattr(inst, "engine", None) is not None:
                head, keep = waits[:-maxw], waits[-maxw:]
                for i in range(0, len(head), maxw):
                    nop = copy.deepcopy(proto(inst.engine))
                    ctr += 1
                    nop.name = f"I-wsplit-{ctr}"
                    nop.sync_info = mybir.SyncInfo(on_wait=head[i:i + maxw], on_update=[])
                    out.append(nop)
                si.on_wait = keep
                changed = True
            out.append(inst)
        if changed:
            blk.instructions = out
    return


DEFAULT_FLAGS = frozenset({"lag3", "dma_split", "rope_all_dve", "fast_norm",
                           "obuf_act", "evict_act", "wide_proj"})


def _build_nc(R=1, phases=(1, 2, 3), flags=DEFAULT_FLAGS):
    import concourse.bass as bass
    import concourse.mybir as mybir
    import concourse.tile as tile

    lag = (4 if "lag4" in flags else 3 if "lag3" in flags
           else 2 if "lag2" in flags else 1)

    f32 = mybir.dt.float32
    f32r = mybir.dt.float32r
    bf16 = mybir.dt.bfloat16
    Exp = mybir.ActivationFunctionType.Exp

    nc = bass.Bass("TRN2", target_bir_lowering=False, debug=False)
    xT = nc.dram_tensor("xT", [D, T], f32r, kind="ExternalInput").ap()
    wqT = nc.dram_tensor("wqT", [D, FS], f32r, kind="ExternalInput").ap()
    wkT = nc.dram_tensor("wkT", [D, FS], f32r, kind="ExternalInput").ap()
    wvT = nc.dram_tensor("wvT", [D, FS], f32r, kind="ExternalInput").ap()
    woT = nc.dram_tensor("woT", [FS, D], f32r, kind="ExternalInput").ap()
    ropeC = nc.dram_tensor("ropeC", [128, T], f32, kind="ExternalInput").ap()
    ropeS = nc.dram_tensor("ropeS", [128, T], f32, kind="ExternalInput").ap()
    triM = nc.dram_tensor("triM", [128, 128], bf16, kind="ExternalInput").ap()
    ident = nc.dram_tensor("ident", [128, 128], bf16, kind="ExternalInput").ap()
    tri = nc.dram_tensor("tri", [128, 128], f32r, kind="ExternalInput").ap()
    triMr = nc.dram_tensor("triMr", [128, 128], f32r, kind="ExternalInput").ap()
    identr = nc.dram_tensor("identr", [128, 128], f32r, kind="ExternalInput").ap()
    out_dt = bf16 if "out_bf16" in flags else f32
    out = nc.dram_tensor("out", [T, D], out_dt, kind="ExternalOutput").ap()

    with tile.TileContext(nc) as tc:
      for _rep in range(R):
        with tc.tile_pool(name="persist", bufs=1) as persist:
            qT = persist.tile([128, FC, T], f32r)      # [feat, tok], 2 heads/chunk
            kT = persist.tile([128, FC, T], f32r)
            v_ext = persist.tile([128, TC, H * HD1], f32r)  # [tok, 8*(64+1)]

            # ---------------- phase 1: projections ----------------
            if 1 in phases:
              with tc.tile_pool(name="xw", bufs=1) as xw:
                xT_s = xw.tile([128, DC, T], f32r)
                xr = xT.rearrange("(c p) t -> p c t", p=128)
                # startup order: first token chunk + first weight stripe reach
                # SBUF before the bulk of xT so the PE starts at ~10us.
                # split across 2 queues.
                if "dma_split" in flags:
                    nc.sync.dma_start(xT_s[:, 0:4, 0:512], xr[:, 0:4, 0:512])
                    nc.sync.dma_start(xT_s[:, 4:8, 0:512], xr[:, 4:8, 0:512])
                else:
                    nc.sync.dma_start(xT_s[:, :, 0:512], xr[:, :, 0:512])

                # q, k first (feeds the rope DVE stream early), then v.
                # single SBUF + single PSUM pool: fewer scope barriers
                with tc.tile_pool(name="p1b", bufs=1) as p1b, \
                     tc.tile_pool(name="pp1", bufs=4, space="PSUM") as pp1:
                    wvp = wstripe = ropep = ptmp = p1b
                    pps_v = pps = pp1
                    stripes = {}
                    wst0 = wstripe.tile([128, DC, 128], f32r, tag="wst", bufs=2)
                    nc.sync.dma_start(
                        wst0, wqT[:, 0:128].rearrange("(c p) m -> p c m", p=128)
                    )
                    stripes[(0, 0)] = wst0
                    rc = ropep.tile([128, T], f32)
                    rs = ropep.tile([128, T], f32)
                    nc.sync.dma_start(rc, ropeC)
                    nc.sync.dma_start(rs, ropeS)
                    for tb in range(1, QB):
                        tsl = slice(tb * 512, (tb + 1) * 512)
                        if "dma_split" in flags:
                            nc.sync.dma_start(xT_s[:, 0:4, tsl], xr[:, 0:4, tsl])
                            nc.sync.dma_start(xT_s[:, 4:8, tsl], xr[:, 4:8, tsl])
                        else:
                            nc.sync.dma_start(xT_s[:, :, tsl], xr[:, :, tsl])
                    wvT_s = wvp.tile([128, DC, FS], f32r)
                    # ones columns of v_ext (head h col 64 at 65h+64)
                    nc.vector.memset(
                        v_ext.rearrange("p c (h e) -> p c h e", e=HD1)[:, :, :, HD:HD1].bitcast(f32),
                        1.0,
                    )

                    blkw = 1024 if "wide_proj" in flags else 512
                    pqk_bufs = 2 if "wide_proj" in flags else 4
                    for dst_i, (dst, w_dram) in enumerate(((qT, wqT), (kT, wkT))):
                        for fc in range(FC):
                            if (dst_i, fc) in stripes:
                                wst = stripes[(dst_i, fc)]
                            else:
                                wst = wstripe.tile([128, DC, 128], f32r, tag="wst", bufs=2)
                                nc.sync.dma_start(
                                    wst,
                                    w_dram[:, fc * 128:(fc + 1) * 128].rearrange("(c p) m -> p c m", p=128),
                                )
                            for tb in range(T // blkw):
                                tsl = slice(tb * blkw, (tb + 1) * blkw)
                                psum = pps.tile([128, blkw], f32, tag="pqk",
                                                bufs=pqk_bufs)
                                # a single matmul may write at most 512 psum
                                # columns; split wide blocks into halves
                                for hw0 in range(0, blkw, 512):
                                    for kc in range(DC):
                                        nc.tensor.matmul(
                                            psum[:, hw0:hw0 + 512],
                                            wst[:, kc, :],
                                            xT_s[:, kc, tb * blkw + hw0:
                                                 tb * blkw + hw0 + 512],
                                            start=(kc == 0), stop=(kc == DC - 1),
                                            skip_group_check=True,
                                        )
                                # rope: evict psum into the destination on
                                # the (idle) ACT, then the rotate-half muls
                                # split across DVE and Pool; final add on Pool.
                                tmp = ptmp.tile([128, blkw], f32r, tag="ropetmp",
                                                bufs=(1 if blkw == 1024 else 2))
                                dsl = dst[:, fc, tsl]
                                nc.scalar.copy(dsl, psum)
                                if "no_rope" in flags:
                                    pass  # timing probe only (wrong numerics)
                                else:
                                    if "rope_all_dve" in flags:
                                        small_engs = (nc.vector, nc.vector)
                                    else:
                                        small_engs = (nc.vector, nc.gpsimd)
                                    for eng, hb in zip(small_engs, (0, 64)):
                                        eng.tensor_mul(
                                            out=tmp[hb:hb + 32, :],
                                            in0=dsl[hb + 32:hb + 64],
                                            in1=rs[hb + 32:hb + 64, tsl],
                                        )
                                        eng.tensor_mul(
                                            out=tmp[hb + 32:hb + 64, :],
                                            in0=dsl[hb:hb + 32],
                                            in1=rs[hb:hb + 32, tsl],
                                        )
                                    nc.vector.tensor_mul(out=dsl, in0=dsl, in1=rc[:, tsl])
                                    add_eng = (nc.gpsimd if "rope_pool" in flags
                                               and "rope_all_dve" not in flags else nc.vector)
                                    add_eng.tensor_add(out=dsl, in0=dsl, in1=tmp)

                    # v: out tile [tok 128, feat 512] = x @ Wv^T
                    # (wv DMA deferred here so startup bandwidth goes to q/k;
                    # split per-d_model-chunk so the 8 queues all pull and the
                    # kc-th matmul only waits for its own chunk)
                    wvr = wvT.rearrange("(c p) m -> p c m", p=128)
                    if "dma_split" in flags:
                        for c in range(DC):
                            nc.sync.dma_start(wvT_s[:, c:c + 1, :], wvr[:, c:c + 1, :])
                    else:
                        nc.sync.dma_start(wvT_s, wvr)
                    for tcv in range(TC):
                        psum = pps_v.tile([128, FS], f32, tag="pv")
                        for kc in range(DC):
                            nc.tensor.matmul(
                                psum,
                                xT_s[:, kc, tcv * 128:(tcv + 1) * 128],
                                wvT_s[:, kc, :],
                                start=(kc == 0), stop=(kc == DC - 1),
                            )
                        if "vext_dve" in flags:
                            # DVE is idle once rope drains (v comes after q/k);
                            # keeping these off ACT clears its runway into the
                            # exp-bound attention phase
                            nc.vector.tensor_copy(
                                out=v_ext.rearrange("p c (h e) -> p c h e", e=HD1)[:, tcv, :, 0:HD],
                                in_=psum.rearrange("p (h e) -> p h e", e=HD),
                            )
                        else:
                            nc.scalar.copy(
                                v_ext.rearrange("p c (h e) -> p c h e", e=HD1)[:, tcv, :, 0:HD],
                                psum.rearrange("p (h e) -> p h e", e=HD),
                            )

            # ---------------- phase 2: attention ----------------
            _ctx_cm = tc.tile_pool(name="ctx", bufs=1)
            ctxp = _ctx_cm.__enter__()
            ctxT = ctxp.tile([128, FC, T], f32r)       # normalized ctx.T
            if 2 in phases:
              with tc.tile_pool(name="p2b", bufs=1) as p2b, \
                 tc.tile_pool(name="pp2", bufs=1, space="PSUM") as pp2:
                maskp = nrm = wop = obuf = p2b
                attnp = p2b
                ps_s = ps_x = ps_c = pp2
                nbuf_at = max(6, lag + 3)
                nbuf_px = 1 if "ps_x1" in flags else 2
                if "ident_mask" in flags:
                    dt_m = f32r if "imask_f32r" in flags else bf16
                    trim = maskp.tile([128, 128], dt_m)
                    nc.sync.dma_start(trim, triMr if dt_m is f32r else triM)
                    idt = maskp.tile([128, 128], dt_m)
                    nc.sync.dma_start(idt, identr if dt_m is f32r else ident)
                else:
                    trim = maskp.tile([128, 128], f32r)
                    nc.sync.dma_start(trim, tri)
                ones64 = maskp.tile([1, 64], f32r)
                nc.vector.memset(ones64.bitcast(f32), 1.0)
                woT_s = wop.tile([128, FC, D], f32r)
                wor = woT.rearrange("(c p) o -> p c o", p=128)
                if "dma_split" in flags:
                    for c in range(FC):
                        nc.sync.dma_start(woT_s[:, c:c + 1, :], wor[:, c:c + 1, :])
                else:
                    nc.sync.dma_start(woT_s, wor)

                # qb outer; two heads' chains interleaved to fill pipeline
                # bubbles; out-projection for this qb's tokens follows so it
                # overlaps the next qb's attention.
                wide = {}  # tco -> accumulating [128, 1024] out tile

                def emit_outproj_chunk(pqb, i):
                    tco = 4 * pqb + i // 2
                    ob = i % 2
                    osl = slice(ob * 512, (ob + 1) * 512)
                    psum = ps_x.tile([128, 512], f32, tag="px", bufs=nbuf_px)
                    for fc in range(FC):
                        nc.tensor.matmul(
                            psum,
                            ctxT[:, fc, tco * 128:(tco + 1) * 128],
                            woT_s[:, fc, osl],
                            start=(fc == 0), stop=(fc == FC - 1),
                        )
                    if "wide_out" in flags:
                        if tco not in wide:
                            ow_t = obuf.tile([128, D], f32, tag="ow", bufs=4)
                            wide[tco] = ow_t
                        ow = wide[tco]
                        cp = nc.scalar.copy if "obuf_act" in flags else (
                            lambda o, p: nc.vector.tensor_copy(out=o, in_=p))
                        cp(ow[:, osl], psum)
                        if ob == 1 and "no_out_dma" not in phases:
                            nc.sync.dma_start(out[tco * 128:(tco + 1) * 128, :], ow)
                            del wide[tco]
                        return
                    if "dma_psum" in flags:
                        if "no_out_dma" not in phases:
                            nc.sync.dma_start(out[tco * 128:(tco + 1) * 128, osl], psum)
                        else:
                            ot = obuf.tile([128, 512], f32, tag="ot", bufs=4)
                            nc.vector.tensor_copy(out=ot, in_=psum)
                    else:
                        ot = obuf.tile([128, 512], out_dt, tag="ot", bufs=4)
                        if "obuf_act" in flags:
                            nc.scalar.copy(ot, psum)
                        else:
                            nc.vector.tensor_copy(out=ot, in_=psum)
                        if "no_out_dma" not in phases:
                            nc.sync.dma_start(out[tco * 128:(tco + 1) * 128, osl], ot)

                prev_qb = None
                for qb in range(QB):
                    qsl = slice(qb * 512, (qb + 1) * 512)
                    nkt = 4 * qb + 4
                    for hp in range(H // 2):
                        pair = (2 * hp, 2 * hp + 1)
                        pctx = {}
                        for h in pair:
                            pctx_h = ps_c.tile([HD1, 512], f32, tag=f"pctx{h % 2}")
                            pctx[h] = pctx_h

                        def emit_av(h, kt2, scs, at):
                            for half in range(2):
                                kt = 2 * kt2 + half
                                sc = scs[half]
                                hsl = slice(half * 512 + sc, (half + 1) * 512)
                                nc.tensor.matmul(
                                    pctx[h][:, sc:],
                                    v_ext[:, kt, h * HD1:(h + 1) * HD1],
                                    at[:, hsl],
                                    start=(kt == 0), stop=(kt == nkt - 1),
                                    skip_group_check=True,
                                )

                        pend = deque()  # (h, kt2, scs, at) awaiting AV
                        for kt2 in range(nkt // 2):
                            for h in pair:
                                chunk, po = h // 2, 64 * (h % 2)
                                qh = qT[po:po + 64, chunk, :]
                                kh = kT[po:po + 64, chunk, :]
                                # two kt tiles share a 2-bank psum: ONE exp
                                # covers both halves (valid column ranges)
                                ps = ps_s.tile([128, 1024], f32, tag="ps", bufs=2)
                                at = attnp.tile([128, 1024], f32r, tag="at", bufs=nbuf_at)
                                scs = []
                                for half in range(2):
                                    kt = 2 * kt2 + half
                                    sc = max(0, 128 * (kt - 4 * qb))
                                    scs.append(sc)
                                    hsl = slice(half * 512 + sc, (half + 1) * 512)
                                    diag = kt >= 4 * qb
                                    use_imask = "ident_mask" in flags and diag
                                    nc.tensor.matmul(
                                        ps[:, hsl],
                                        kh[:, kt * 128:(kt + 1) * 128],
                                        qh[:, qb * 512 + sc:(qb + 1) * 512],
                                        start=True, stop=not use_imask,
                                        skip_group_check=True,
                                    )
                                    if use_imask:
                                        # accumulate the -1e9 causal triangle
                                        # into the diagonal 128-col segment:
                                        # I^T @ triM == triM
                                        msl = slice(half * 512 + sc, half * 512 + sc + 128)
                                        nc.tensor.matmul(
                                            ps[:, msl], idt, trim,
                                            start=False, stop=True,
                                            skip_group_check=True,
                                        )
                                while len(pend) > lag:
                                    emit_av(*pend.popleft())
                                # exp over the union width (equal 2-seg AP);
                                # half B's extra low columns are never read.
                                sc0 = scs[0]
                                if sc0 == 0:
                                    nc.scalar.activation(at, ps, Exp, scale=float(SCALE))
                                else:
                                    seg_o = at.rearrange("p (s c) -> p s c", s=2)[:, :, sc0:]
                                    seg_i = ps.rearrange("p (s c) -> p s c", s=2)[:, :, sc0:]
                                    nc.scalar.activation(seg_o, seg_i, Exp, scale=float(SCALE))
                                if "ident_mask" not in flags:
                                    mask_eng = nc.gpsimd if "pool_mask" in flags else nc.vector
                                    for half in range(2):
                                        kt = 2 * kt2 + half
                                        if kt >= 4 * qb:
                                            sc = scs[half]
                                            msl = slice(half * 512 + sc, half * 512 + sc + 128)
                                            mask_eng.tensor_mul(
                                                out=at[:, msl], in0=at[:, msl], in1=trim)
                                pend.append((h, kt2, scs, at))
                        while pend:
                            emit_av(*pend.popleft())

                        # reciprocals right after the last AV; the previous
                        # qb's out-projection matmuls keep the PE busy while
                        # they complete.
                        rrows = {}
                        ctxus = {}
                        for h in pair:
                            rrow = nrm.tile([1, 512], f32r, tag="rrow", bufs=3)
                            with nc.allow_low_precision(reason="recip feeds PE bcast"):
                                nc.vector.reciprocal(rrow, pctx[h][HD:HD1, :])
                            rrows[h] = rrow
                            if "fast_norm" in flags:
                                # evict unnormalized ctx now so the pctx bank
                                # frees for the next pair's first AV without
                                # waiting on the whole normalize chain
                                ctxu = nrm.tile([HD, 512], f32, tag="ctxu", bufs=3)
                                if "evict_act" in flags:
                                    nc.scalar.copy(ctxu, pctx[h][0:HD, :])
                                else:
                                    nc.vector.tensor_copy(out=ctxu, in_=pctx[h][0:HD, :])
                                ctxus[h] = ctxu
                        if 3 in phases and prev_qb is not None:
                            emit_outproj_chunk(prev_qb, 2 * hp)
                            emit_outproj_chunk(prev_qb, 2 * hp + 1)
                        for h in pair:
                            chunk, po = h // 2, 64 * (h % 2)
                            rbp = ps_x.tile([64, 512], f32, tag="px", bufs=nbuf_px)
                            nc.tensor.matmul(rbp, ones64, rrows[h], start=True, stop=True)
                            if "fast_norm" in flags:
                                # SBUF ctx * PSUM broadcast: one psum operand
                                nc.vector.tensor_mul(
                                    out=ctxT[po:po + 64, chunk, qsl],
                                    in0=ctxus[h],
                                    in1=rbp,
                                )
                            else:
                                rb = nrm.tile([64, 512], f32, tag="rb", bufs=3)
                                nc.vector.tensor_copy(out=rb, in_=rbp)
                                nc.vector.tensor_mul(
                                    out=ctxT[po:po + 64, chunk, qsl],
                                    in0=pctx[h][0:HD, :],
                                    in1=rb,
                                )
                    prev_qb = qb
                if 3 in phases:
                    for i in range(8):
                        emit_outproj_chunk(QB - 1, i)
            _ctx_cm.__exit__(None, None, None)

    _split_waits(nc, mybir)
    return nc


def _make_runner(nc, n_cores):
    """Build the shard_map-jitted PJRT executable once (reusable across calls)."""
    import jax
    import concourse.mybir as mybir
    from jax.sharding import Mesh, PartitionSpec
    from jax.experimental.shard_map import shard_map
    from concourse import bass2jax as b2j

    b2j.install_neuronx_cc_hook()
    partition_name = nc.partition_id_tensor.name if nc.partition_id_tensor else None
    in_names, out_names, out_avals = [], [], []
    for alloc in nc.m.functions[0].allocations:
        if not isinstance(alloc, mybir.MemoryLocationSet):
            continue
        name = alloc.memorylocations[0].name
        if alloc.kind == "ExternalInput":
            if name != partition_name:
                in_names.append(name)
        elif alloc.kind == "ExternalOutput":
            out_names.append(name)
            out_avals.append(
                jax.core.ShapedArray(tuple(alloc.tensor_shape), mybir.dt.np(alloc.dtype))
            )
    all_in_names = list(in_names) + list(out_names)
    if partition_name is not None:
        all_in_names.append(partition_name)

    def _body(*args):
        operands = list(args)
        if partition_name is not None:
            operands.append(b2j.partition_id_tensor())
        return tuple(b2j._bass_exec_p.bind(
            *operands,
            out_avals=tuple(out_avals),
            in_names=tuple(all_in_names),
            out_names=tuple(out_names),
            lowering_input_output_aliases=(),
            sim_require_finite=True,
            sim_require_nnan=True,
            nc=nc,
        ))

    devices = jax.devices()[:n_cores]
    mesh = Mesh(np.asarray(devices), ("core",))
    n_in = len(in_names) + len(out_names)
    fn = jax.jit(
        shard_map(
            _body, mesh=mesh,
            in_specs=(PartitionSpec("core"),) * n_in,
            out_specs=(PartitionSpec("core"),) * len(out_names),
            check_rep=False,
        ),
        keep_unused=True,
    )

    def stage(in_maps):
        import jax as _jax
        per_core = [[np.asarray(m[name]) for name in in_names] for m in in_maps]
        concat_in = [
            np.concatenate([per_core[c][i] for c in range(n_cores)], axis=0)
            for i in range(len(in_names))
        ]
        concat_zeros = [
            np.zeros((n_cores * a.shape[0], *a.shape[1:]), a.dtype) for a in out_avals
        ]
        return [_jax.device_put(a) for a in concat_in + concat_zeros]

    def call_staged(staged):
        import jax as _jax
        out_arrs = fn(*staged)
        _jax.block_until_ready(out_arrs)
        return out_arrs

    def call(in_maps):
        out_arrs = call_staged(stage(in_maps))
        return [
            {name: np.asarray(out_arrs[i]).reshape(n_cores, *out_avals[i].shape)[c]
             for i, name in enumerate(out_names)}
            for c in range(n_cores)
        ]

    call.stage = stage
    call.call_staged = call_staged
    return call


def _host_tables():
    # rope tables in the permuted ([even dims | odd dims] per head) layout:
    # rows 0:32 -> freq j (x1 of head A), 32:64 -> freq j (x2 of head A), repeat.
    j = np.arange(32, dtype=np.float64)
    inv = 1.0 / (10000.0 ** (2.0 * j / HD))
    t = np.arange(T, dtype=np.float64)
    ang = np.outer(inv, t)                      # [32, T]
    c32 = np.cos(ang).astype(np.float32)
    s32 = np.sin(ang).astype(np.float32)
    ropeC = np.concatenate([c32, c32, c32, c32], axis=0)          # [128, T]
    ropeS = np.concatenate([s32, -s32, s32, -s32], axis=0)        # [128, T]
    # additive causal mask for the [128,128] diagonal tile:
    # 0 where key p <= query f (valid), -1e9 otherwise
    import ml_dtypes
    p = np.arange(128)[:, None]
    f = np.arange(128)[None, :]
    triM = np.where(p <= f, 0.0, -1e9).astype(ml_dtypes.bfloat16)
    ident = np.eye(128).astype(ml_dtypes.bfloat16)
    tri = (p <= f).astype(np.float32)
    triMr = np.where(p <= f, 0.0, -1e9).astype(np.float32)
    identr = np.eye(128, dtype=np.float32)
    return ropeC, ropeS, triM, ident, tri, triMr, identr


def _perm_rows():
    # per head: [even dims, odd dims]
    perm = []
    for h in range(H):
        base = h * HD
        perm.extend(base + np.arange(0, HD, 2))
        perm.extend(base + np.arange(1, HD, 2))
    return np.asarray(perm)


def _in_maps(x, Wq, Wk, Wv, Wo):
    ropeC, ropeS, triM, ident, tri, triMr, identr = _host_tables()
    perm = _perm_rows()
    in_maps = []
    for c in range(N_CORES):
        b, r = c // 2, c % 2
        rows = slice(r * FS, (r + 1) * FS)
        in_maps.append({
            "xT": np.ascontiguousarray(x[b].T),
            "wqT": np.ascontiguousarray(Wq[rows][perm].T),
            "wkT": np.ascontiguousarray(Wk[rows][perm].T),
            "wvT": np.ascontiguousarray(Wv[rows].T),
            "woT": np.ascontiguousarray(Wo[:, rows].T),
            "ropeC": ropeC,
            "ropeS": ropeS,
            "triM": triM,
            "ident": ident,
            "tri": tri,
            "triMr": triMr,
            "identr": identr,
        })
    return in_maps


def kernel(x, Wq, Wk, Wv, Wo):
    x = np.asarray(x, dtype=np.float32)
    Wq = np.asarray(Wq, dtype=np.float32)
    Wk = np.asarray(Wk, dtype=np.float32)
    Wv = np.asarray(Wv, dtype=np.float32)
    Wo = np.asarray(Wo, dtype=np.float32)

    if "runner" not in _CACHE:
        nc = _build_nc()
        _CACHE["runner"] = _make_runner(nc, N_CORES)
    call = _CACHE["runner"]

    results = call(_in_maps(x, Wq, Wk, Wv, Wo))
    out = np.empty((B, T, D), dtype=np.float32)
    for b in range(B):
        out[b] = (np.asarray(results[2 * b]["out"], dtype=np.float32)
                  + np.asarray(results[2 * b + 1]["out"], dtype=np.float32))
    return out
